# revision 38
# baseline (speedup 1.0000x reference)
"""Causal attention with RoPE, tensor-parallel over 8 NeuronCores. v6.

Problem: B=2, L=2048, d_model=2048, H=16 heads, D=128 head dim.
  qkv = X @ W_qkv  (per-head [q|k|v] column layout)
  Q,K rope'd (interleaved pairs), causal softmax(QK^T/sqrt(D)) @ V, @ W_out.

Sharding (Megatron-style): core c handles batch b=c//4 and head group
g=c%4 (4 heads). Each core computes a partial output; host sums 4
partials per batch.

v6 strategy: hybrid precision keyed on causal key count. Softmax-weight
noise from fp8 quantization only survives in rows with few keys (no
averaging), so the first 512-row chunk runs an exact path (3-term fp8
projections, bf16 S/E/O, 3-term out-proj) while chunks 1-3 run
aggressive fp8:
  - projections: 1-term plain-fp8 (hi only)
  - RoPE fold: one fp8 DR matmul (hi-only u/w planes)
  - S^T blocks: one fp8 DR matmul per block, stationary = (K_hi, zeros);
    the dead second plane means no Q duplication (both zero-planes are
    memset once so 0*garbage can't produce NaN)
  - E = exp(S/sqrt(D) - 1.25) stored fp8 (bias keeps exp <= 448 and
    cancels exactly in the softmax ratio)
  - O^T accumulation: fp8 DR over adjacent l-tile pairs
  - out-proj: 1-term (O_hi @ Wout_hi)
Numpy-simulated end-to-end metric ~1.06e-2 vs the 2e-2 gate.

Scheduling: engines execute their streams in emission order, so phases
must be interleaved at emission time. Projection chunk lc=c+1 and
out-proj chunk c-1 are generators pumped as PE filler inside attention
chunk c's ACT-bound exp stream. PSUM is one pool: pss 2x(2 banks,
S-pairs + pso), ps 2x(proj + out-proj accumulators), pr 1 (fold),
dn 1 (denominator chain) = 8 banks.
"""
import math
import numpy as np
import ml_dtypes
import concourse.bacc as bacc
import concourse.mybir as mybir
import concourse.tile as tile
from concourse.bass_utils import run_bass_kernel_spmd

F32 = mybir.dt.float32
F32R = mybir.dt.float32r
BF16 = mybir.dt.bfloat16
F8 = mybir.dt.float8e4
AF = mybir.ActivationFunctionType
DR = mybir.MatmulPerfMode.DoubleRow

N_HEADS = 16
D = 128
THETA = 10000.0
B_FULL, L_FULL, DM_FULL = 2, 2048, 2048
H_PER_CORE = 4
N_CORES = 8
NP_F8 = ml_dtypes.float8_e4m3
NP_BF = ml_dtypes.bfloat16
WSC = 32.0   # W-lo residual pre-scale
EB = -1.25   # exp bias: keeps fp8 E in range; cancels in softmax ratio


def build_nc(L=L_FULL, DM=DM_FULL, H=H_PER_CORE):
    LT = L // 128           # l-tiles (16)
    KT = DM // 128          # contract tiles for projections (16)
    KP = KT // 2            # contract tile PAIRS for DoubleRow (8)
    HD = H * D              # qkv width per core (512)
    QC = L // 512           # q-chunks (4)
    OC = DM // 512          # out-proj n-chunks (4)
    ISQ = 1.0 / math.sqrt(D)

    nc = bacc.Bacc(None, target_bir_lowering=False)

    xh_d = nc.dram_tensor("x8h", [128, QC, KT, 512], F8, kind="ExternalInput")
    xl_d = nc.dram_tensor("x8l", [128, KT, 512], F8, kind="ExternalInput")
    xs_d = nc.dram_tensor("x8s", [128, KT, 512], F8, kind="ExternalInput")
    w8_d = {}
    for w in ("wq", "wk"):
        for p in ("h", "l"):    # hi, 32*lo
            w8_d[w + p] = nc.dram_tensor(f"{w}8{p}", [128, H, KT, D], F8,
                                         kind="ExternalInput")
    for p in ("h", "l"):
        w8_d["wv" + p] = nc.dram_tensor(f"wv8{p}", [128, KT, HD], F8,
                                        kind="ExternalInput")
    cs1_d = nc.dram_tensor("cs1", [128, L], F32, kind="ExternalInput")
    cs2_d = nc.dram_tensor("cs2", [128, L], F32, kind="ExternalInput")
    pmat_d = nc.dram_tensor("pmat", [128, 2, 128], F32R,
                            kind="ExternalInput")
    pmat8_d = nc.dram_tensor("pmat8", [128, 2, 128], F8,
                             kind="ExternalInput")
    mask_d = nc.dram_tensor("mask128", [128, 128], BF16, kind="ExternalInput")
    mask2_d = nc.dram_tensor("mask256", [128, 256], BF16,
                             kind="ExternalInput")
    mask8_d = nc.dram_tensor("mask128f8", [128, 128], F8,
                             kind="ExternalInput")
    mask28_d = nc.dram_tensor("mask256f8", [128, 256], F8,
                              kind="ExternalInput")
    idm_d = nc.dram_tensor("idm", [128, 128], F32, kind="ExternalInput")
    sel_d = nc.dram_tensor("selmat", [4, 512], BF16, kind="ExternalInput")
    wo8h_d = nc.dram_tensor("wo8h", [128, H, DM], F8, kind="ExternalInput")
    wo8l_d = nc.dram_tensor("wo8l", [128, H, DM], F8, kind="ExternalInput")
    y_d = nc.dram_tensor("y", [L, DM], F32, kind="ExternalOutput")

    with (
        tile.TileContext(nc) as tc,
        tc.tile_pool(name="res", bufs=1) as resp,
        tc.tile_pool(name="uw", bufs=2) as uwp,
        tc.tile_pool(name="ot", bufs=2) as otp,
        tc.tile_pool(name="dn", bufs=2) as dnp,
        tc.tile_pool(name="psm", bufs=1, space="PSUM") as psm,
    ):
        # ---- resident tiles ----
        qt8 = resp.tile([128, 2, H, L], F8)     # plane1 = zeros
        kt8 = resp.tile([128, 2, H, L], F8)     # plane1 = zeros
        v8 = resp.tile([128, LT, HD], F8)       # V [l,d] fp8
        pmat = resp.tile([128, 2, 128], F32R)
        pmat8 = resp.tile([128, 2, 128], F8)
        mask = resp.tile([128, 128], BF16)
        mask2 = resp.tile([128, 256], BF16)
        mask8 = resp.tile([128, 128], F8)
        mask28 = resp.tile([128, 256], F8)
        idm = resp.tile([128, 128], F32)
        selm = resp.tile([4, 512], BF16)
        ones = resp.tile([128, 1], BF16)
        ones8 = resp.tile([128, 1], F8)
        ebt = resp.tile([128, 1], F32)
        cs1 = resp.tile([128, L], F32)
        cs2 = resp.tile([128, L], F32)
        x3h = resp.tile([128, KT, 512], F8)
        xh1 = resp.tile([128, KT, 512], F8)
        xh2 = resp.tile([128, KT, 512], F8)
        wv8h_r = resp.tile([128, KT, HD], F8)
        wq8h = resp.tile([128, H, KT, D], F8)
        wk8h = resp.tile([128, H, KT, D], F8)
        nc.vector.memset(ones[:], 1.0)
        nc.vector.memset(ones8[:], 1.0)
        nc.vector.memset(ebt[:], EB)
        # dead DR planes must be defined (0 * NaN would poison S).
        # Split across DVE (idle at t0) and Pool so the Pool queue
        # reaches the diagonal masks sooner.
        nc.vector.memset(qt8[:, 1], 0.0)
        nc.gpsimd.memset(kt8[:, 1], 0.0)

        # ---- scoped pools (closed mid-build, LIFO, to recycle SBUF) ----
        earlyp_cm = tc.tile_pool(name="early", bufs=1)
        earlyp = earlyp_cm.__enter__()
        lc0p_cm = tc.tile_pool(name="lc0x", bufs=1)
        lc0p = lc0p_cm.__enter__()
        qtbf = earlyp.tile([128, H, 512], BF16)   # chunk-0 Q (bf16)
        ktbf = earlyp.tile([128, H, 512], BF16)   # K tiles 0-3 (bf16)
        vbf = earlyp.tile([128, 4, HD], BF16)     # V tiles 0-3 (bf16)
        x0h = lc0p.tile([128, KT, 512], F8)
        x0l = lc0p.tile([128, KT, 512], F8)
        x0s = lc0p.tile([128, KT, 512], F8)
        wq8l = lc0p.tile([128, H, KT, D], F8)
        wk8l = lc0p.tile([128, H, KT, D], F8)
        wv8l = lc0p.tile([128, KT, HD], F8)
        P = {}   # late-opened pool handle (es8 / wo8 / ysb)

        # ---- cold-start DMAs: first-needed-first on two HWDGE queues ----
        _alt = [0]

        def dma(dst, src):
            eng = (nc.sync, nc.scalar)[_alt[0] % 2]
            _alt[0] += 1
            eng.dma_start(dst, src)

        for h in range(H):
            dma(wq8h[:, h], w8_d["wqh"][:, h])
            dma(x0h[:, 4 * h:4 * (h + 1)], xh_d[:, 0, 4 * h:4 * (h + 1)])
        for q in range(2):
            dma(x0l[:, 8 * q:8 * (q + 1)], xl_d[:, 8 * q:8 * (q + 1)])
        for h in range(H):
            dma(wq8l[:, h], w8_d["wql"][:, h])
            dma(x0s[:, 4 * h:4 * (h + 1)], xs_d[:, 4 * h:4 * (h + 1)])
        dma(cs1[:, 0:512], cs1_d[:, 0:512])
        dma(cs2[:, 0:512], cs2_d[:, 0:512])
        dma(pmat[:], pmat_d[:, :])
        dma(pmat8[:], pmat8_d[:, :])
        for h in range(H):
            dma(wk8h[:, h], w8_d["wkh"][:, h])
        dma(xh1[:, 0:8], xh_d[:, 1, 0:8])
        dma(xh1[:, 8:], xh_d[:, 1, 8:])
        dma(wk8l[:], w8_d["wkl"][:, :])
        dma(wv8h_r[:], w8_d["wvh"][:, :])
        dma(wv8l[:], w8_d["wvl"][:, :])
        dma(cs1[:, 512:], cs1_d[:, 512:])
        dma(cs2[:, 512:], cs2_d[:, 512:])
        dma(mask[:], mask_d[:, :])
        dma(mask2[:], mask2_d[:, :])
        dma(mask8[:], mask8_d[:, :])
        dma(mask28[:], mask28_d[:, :])
        dma(idm[:], idm_d[:, :])
        dma(selm[:], sel_d[:, :])
        nc.sync.dma_start(xh2[:], xh_d[:, 2])
        nc.sync.dma_start(x3h[:], xh_d[:, 3])
        xh_t = {1: xh1, 2: xh2, 3: x3h}

        # ---- PSUM tile helpers ----
        def ps_tile(name):
            return psm.tile([128, 512], F32, tag="ps", bufs=2, name=name)

        def pss_tile(name, shape=(128, 2, 512)):
            return psm.tile(list(shape), F32, tag="pss", bufs=2, name=name)

        def pr_tile(name):
            # fold outputs share the ps ring (2 slots cover the proj
            # ps->fold->store pipeline)
            return psm.tile([128, 512], F32, tag="ps", bufs=2, name=name)

        def dn_tile(name, shape):
            # 2 slots so unit n+1's denominator colsums don't serialize
            # behind unit n's den-chain drain
            return psm.tile(list(shape), F32, tag="dn", bufs=2, name=name)

        # ---- lc0 projections: 3-term, software-pipelined ----
        pend = [None]   # (kind, h, ps)

        def store_late(kind, h, lc, pr):
            sl = slice(512 * lc, 512 * (lc + 1))
            if kind == "q":
                nc.scalar.activation(qt8[:, 0, h, sl], pr[:], AF.Copy)
            else:
                nc.vector.tensor_copy(kt8[:, 0, h, sl], pr[:])

        def flush0(nxt):
            if pend[0] is not None:
                kind, h, ps = pend[0]
                if kind == "v":
                    nc.scalar.activation(vbf[:, h, :], ps[:], AF.Copy)
                    nc.gpsimd.tensor_copy(v8[:, h, :], vbf[:, h, :])
                else:
                    u = uwp.tile([128, 512], F32R, tag="u")
                    w = uwp.tile([128, 512], F32R, tag="w")
                    nc.vector.tensor_mul(u[:], ps[:], cs1[:, 0:512])
                    nc.vector.tensor_mul(w[:], ps[:], cs2[:, 0:512])
                    pr = pr_tile(f"pr0{kind}{h}")
                    nc.tensor.matmul(pr[:], pmat[:, 0], u[:],
                                     start=True, stop=False,
                                     skip_group_check=True)
                    nc.tensor.matmul(pr[:], pmat[:, 1], w[:],
                                     start=False, stop=True,
                                     skip_group_check=True)
                    if kind == "q":
                        nc.scalar.activation(qtbf[:, h, :], pr[:], AF.Copy)
                    else:
                        nc.scalar.activation(ktbf[:, h, :], pr[:], AF.Copy)
                        nc.gpsimd.tensor_copy(kt8[:, 0, h, 0:512],
                                              ktbf[:, h, :])
            pend[0] = nxt

        for wn, wh_t, wl_t, kind in (("wq", wq8h, wq8l, "q"),
                                     ("wk", wk8h, wk8l, "k")):
            terms = ((wh_t, x0h), (wh_t, x0l), (wl_t, x0s))
            if kind == "q":
                # term-major: fill the PE while late DMAs land. t1/t2
                # j-outer (matches X half arrival), t3 h-outer (matches
                # per-head wql arrival). qps h=2,3 borrow pss slots so
                # the ps ring stays at 2 bufs.
                qps = [ps_tile(f"q0ps{h}") if h < 2 else
                       pss_tile(f"q0ps{h}", (128, 512)) for h in range(H)]
                for t, (ww, xx) in enumerate(terms):
                    hj = ([(h, j) for j in range(KP) for h in range(H)]
                          if t < 2 else
                          [(h, j) for h in range(H) for j in range(KP)])
                    for h, j in hj:
                        jsl = slice(2 * j, 2 * j + 2)
                        nc.tensor.matmul(qps[h][:], ww[:, h, jsl],
                                         xx[:, jsl],
                                         start=(t == 0 and j == 0),
                                         stop=(t == 2 and j == KP - 1),
                                         perf_mode=DR)
                for h in range(H):
                    flush0((kind, h, qps[h]))
                continue
            for h in range(H):
                ps = ps_tile(f"k0ps{h}")
                for t, (ww, xx) in enumerate(terms):
                    for j in range(KP):
                        jsl = slice(2 * j, 2 * j + 2)
                        nc.tensor.matmul(ps[:], ww[:, h, jsl], xx[:, jsl],
                                         start=(t == 0 and j == 0),
                                         stop=(t == 2 and j == KP - 1),
                                         perf_mode=DR)
                flush0((kind, h, ps))
        for li in range(4):
            ps = ps_tile(f"v0ps{li}")
            lsl = slice(128 * li, 128 * (li + 1))
            vterms = ((x0h, wv8h_r), (x0l, wv8h_r), (x0s, wv8l))
            for t, (xx, ww) in enumerate(vterms):
                for j in range(KP):
                    jsl = slice(2 * j, 2 * j + 2)
                    nc.tensor.matmul(ps[:], xx[:, jsl, lsl], ww[:, jsl],
                                     start=(t == 0 and j == 0),
                                     stop=(t == 2 and j == KP - 1),
                                     perf_mode=DR)
            flush0(("v", li, ps))
        flush0(None)
        lc0p_cm.__exit__(None, None, None)

        # ---- filler machinery ----
        class G:
            """Generator with progress tracking for dependency gates."""
            def __init__(self, gen):
                self.gen = gen
                self.steps = 0
                self.done = False

            def step(self):
                if self.done:
                    return False
                try:
                    next(self.gen)
                    self.steps += 1
                    return True
                except StopIteration:
                    self.done = True
                    return False

        filler = []

        def pump(k):
            n = 0
            while filler and n < k:
                g = filler[0]
                if g.step():
                    n += 1
                else:
                    filler.remove(g)

        def gate(g, steps):
            while g is not None and not g.done and g.steps < steps:
                g.step()
            if g is not None and g.done and g in filler:
                filler.remove(g)

        def proj_gen(lc):
            """1-term projections + fp8 folds for chunk lc (1..3).

            Unit order q-h0, k-h0, v-0..3, then (q,k) per remaining head
            so attention unit (lc, h) can gate on a step prefix:
            h0 after 58 steps, then +22 per head.
            """
            xh = xh_t[lc]
            sl = slice(512 * lc, 512 * (lc + 1))

            def qk_unit(wh_t, kind, h):
                ps = ps_tile(f"p{lc}{kind}{h}")
                for j in range(KP):
                    jsl = slice(2 * j, 2 * j + 2)
                    nc.tensor.matmul(ps[:], wh_t[:, h, jsl], xh[:, jsl],
                                     start=(j == 0), stop=(j == KP - 1),
                                     perf_mode=DR)
                    yield
                uw8 = uwp.tile([128, 2, 512], F8, tag="uw8")
                nc.vector.tensor_mul(uw8[:, 0], ps[:], cs1[:, sl])
                nc.vector.tensor_mul(uw8[:, 1], ps[:], cs2[:, sl])
                yield
                pr = pr_tile(f"pr{lc}{kind}{h}")
                nc.tensor.matmul(pr[:], pmat8[:, :], uw8[:],
                                 start=True, stop=True, perf_mode=DR)
                yield
                store_late(kind, h, lc, pr)
                yield

            def v_unit(li):
                ps = ps_tile(f"p{lc}v{li}")
                lsl = slice(128 * li, 128 * (li + 1))
                for j in range(KP):
                    jsl = slice(2 * j, 2 * j + 2)
                    nc.tensor.matmul(ps[:], xh[:, jsl, lsl],
                                     wv8h_r[:, jsl],
                                     start=(j == 0), stop=(j == KP - 1),
                                     perf_mode=DR)
                    yield
                nc.scalar.activation(v8[:, 4 * lc + li, :], ps[:], AF.Copy)
                yield

            yield from qk_unit(wq8h, "q", 0)
            yield from qk_unit(wk8h, "k", 0)
            for li in range(4):
                yield from v_unit(li)
            for h in range(1, H):
                yield from qk_unit(wq8h, "q", h)
                yield from qk_unit(wk8h, "k", h)

        # prefix of proj_gen needed before attention unit (c, h)
        PJ_NEED = [58, 80, 102, 124]

        def out_proj_gen(c, oh8, ol8, os8):
            # chunk 0: 3-term; chunks 1-3: 1-term (hi only)
            if c == 0:
                yterms = ((oh8, P["wo8h"]), (ol8, P["wo8h"]),
                          (os8, P["wo8l"]))
            else:
                yterms = ((oh8, P["wo8h"]),)
            nt = len(yterms)
            for li in range(4):
                i = 4 * c + li
                lsl = slice(128 * li, 128 * (li + 1))
                ysb = P["yp"].tile([128, OC, 512], F32, tag="ysb", bufs=3)
                for o in range(OC):
                    osl = slice(512 * o, 512 * (o + 1))
                    psy = ps_tile(f"y{c}{li}{o}")
                    for t, (oo, ww) in enumerate(yterms):
                        for hp in range(H // 2):
                            h2 = slice(2 * hp, 2 * hp + 2)
                            nc.tensor.matmul(
                                psy[:], oo[:, h2, lsl], ww[:, h2, osl],
                                start=(t == 0 and hp == 0),
                                stop=(t == nt - 1 and hp == H // 2 - 1),
                                perf_mode=DR)
                            yield
                    if o == 2 or (c == QC - 1 and o == 0):
                        nc.scalar.activation(ysb[:, o, :], psy[:], AF.Copy)
                    else:
                        nc.vector.tensor_copy(ysb[:, o, :], psy[:])
                    yield
                nc.sync.dma_start(y_d[128 * i:128 * (i + 1), :],
                                  ysb[:, :, :])
                yield

        def den_tail_gen(pso, psden, h, c, oh8, ol8, os8):
            # deferred den-chain: PE ops interleave with the NEXT unit's
            # S blocks so the DVE reciprocal never gates PE
            inv = dnp.tile([128, 4], F32, tag="inv")
            nc.vector.reciprocal(inv[:], psden[:])
            yield
            pit = dn_tile("pit", (4, 128))
            nc.tensor.transpose(pit[:], inv[:], idm[:])
            invt = dnp.tile([4, 128], BF16, tag="invt")
            nc.vector.tensor_copy(invt[:], pit[:])
            yield
            bc = dn_tile("bc", (128, 512))
            for b in range(4):
                bsl = slice(128 * b, 128 * (b + 1))
                nc.tensor.matmul(bc[:, bsl], selm[:, bsl], invt[:],
                                 start=True, stop=True)
            yield
            ot = otp.tile([128, 512], F32R, tag="ot")
            nc.vector.tensor_mul(ot[:], pso[:], bc[:])
            nc.gpsimd.tensor_copy(oh8[:, h, :], ot[:])
            yield
            if c == 0:
                nc.gpsimd.tensor_sub(ol8[:, h, :], ot[:], oh8[:, h, :])
                nc.gpsimd.tensor_scalar_mul(os8[:, h, :], ot[:],
                                            1.0 / WSC)
                yield

        def drive(tail, n):
            if tail is not None:
                try:
                    next(tail)
                    n -= 1
                except StopIteration:
                    tail = None
            pump(n)
            return tail

        def attn_unit0(h, oh8, ol8, os8, tail):
            # chunk 0: exact bf16 path
            es = earlyp.tile([128, 4, 512], BF16, tag="esb", bufs=2)
            for p in range(2):
                pss = pss_tile(f"s0{h}{p}")
                for mm_ in range(2):
                    m = 2 * p + mm_
                    qo = 128 * m if m > 0 else 0
                    nc.tensor.matmul(pss[:, mm_, qo:],
                                     ktbf[:, h, 128 * m:128 * (m + 1)],
                                     qtbf[:, h, qo:],
                                     start=True, stop=True)
                    tail = drive(tail, 4)
                qp = 256 * p
                nc.scalar.activation(es[:, 2 * p:2 * p + 2, qp:],
                                     pss[:, :, qp:], AF.Exp, scale=ISQ)
                nc.gpsimd.tensor_mul(es[:, 2 * p, qp:qp + 128],
                                     es[:, 2 * p, qp:qp + 128], mask[:])
                nc.gpsimd.tensor_mul(es[:, 2 * p + 1, qp:qp + 256],
                                     es[:, 2 * p + 1, qp:qp + 256],
                                     mask2[:])
            while tail is not None:
                tail = drive(tail, 0)
            pso = pss_tile(f"o0{h}", (128, 512))
            psden = dn_tile("psden", (128, 4))
            for m in range(4):
                qo = 128 * m if m > 0 else 0
                nc.tensor.matmul(pso[:, qo:],
                                 vbf[:, m, 128 * h:128 * (h + 1)],
                                 es[:, m, qo:],
                                 start=(m == 0), stop=(m == 3))
                for b in range(max(0, m), 4):
                    nc.tensor.matmul(psden[:, b:b + 1],
                                     es[:, m, 128 * b:128 * (b + 1)],
                                     ones[:],
                                     start=(m == 0 and b == 0),
                                     stop=(m == 3 and b == 3),
                                     skip_group_check=True)
                pump(2)
            osb = otp.tile([128, 512], F32R, tag="osb")
            nc.vector.tensor_copy(osb[:], pso[:])
            return den_tail_gen(osb, psden, h, 0, oh8, ol8, os8)

        def attn_unit(h, c, oh8, ol8, os8, tail):
            # chunks 1-3: fp8 DR path
            nblk = 4 * (c + 1)
            es = P["ep"].tile([128, LT, 512], F8, tag="es", bufs=2)
            for jp in range(nblk // 2):
                pss = pss_tile(f"s{c}{h}{jp}")
                for jj in range(2):
                    j = 2 * jp + jj
                    m = j - 4 * c
                    qo = 128 * m if m > 0 else 0
                    nc.tensor.matmul(
                        pss[:, jj, qo:],
                        kt8[:, :, h, 128 * j:128 * (j + 1)],
                        qt8[:, :, h, 512 * c + qo:512 * (c + 1)],
                        start=True, stop=True, perf_mode=DR)
                    tail = drive(tail, 4)
                j0 = 2 * jp
                m0 = j0 - 4 * c
                qp = 128 * m0 if m0 > 0 else 0
                nc.scalar.activation(es[:, j0:j0 + 2, qp:],
                                     pss[:, :, qp:], AF.Exp,
                                     scale=ISQ, bias=ebt[:])
                if m0 >= 0:
                    nc.gpsimd.tensor_mul(es[:, j0, qp:qp + 128],
                                         es[:, j0, qp:qp + 128], mask8[:])
                    nc.gpsimd.tensor_mul(es[:, j0 + 1, qp:qp + 256],
                                         es[:, j0 + 1, qp:qp + 256],
                                         mask28[:])
            while tail is not None:
                tail = drive(tail, 0)
            # O^T accumulation (fp8 DR over l-tile pairs) + denominator
            # colsum matmuls
            pso = pss_tile(f"o{c}{h}", (128, 512))
            psden = dn_tile("psden", (128, 4))
            hsl = slice(128 * h, 128 * (h + 1))
            for jp in range(nblk // 2):
                j0 = 2 * jp
                m0 = j0 - 4 * c
                qo = 128 * m0 if m0 > 0 else 0
                nc.tensor.matmul(pso[:, qo:],
                                 v8[:, j0:j0 + 2, hsl],
                                 es[:, j0:j0 + 2, qo:],
                                 start=(jp == 0),
                                 stop=(jp == nblk // 2 - 1),
                                 perf_mode=DR)
                for jj in range(2):
                    j = j0 + jj
                    m = j - 4 * c
                    for b in range(max(0, m), 4):
                        nc.tensor.matmul(psden[:, b:b + 1],
                                         es[:, j, 128 * b:128 * (b + 1)],
                                         ones8[:],
                                         start=(j == 0 and b == 0),
                                         stop=(j == nblk - 1 and b == 3),
                                         skip_group_check=True)
                pump(2)
            osb = otp.tile([128, 512], F32R, tag="osb")
            nc.vector.tensor_copy(osb[:], pso[:])
            return den_tail_gen(osb, psden, h, c, oh8, ol8, os8)

        # ---- chunk loop ----
        pending = None
        tail = None
        pgs = {}
        for c in range(QC):
            if c == 1:
                # recycle chunk-0 SBUF for the late-chunk pools
                earlyp_cm.__exit__(None, None, None)
                latep_cm = tc.tile_pool(name="late", bufs=1)
                latep = latep_cm.__enter__()
                P["cm"] = latep_cm
                P["ep"] = latep
                P["yp"] = latep
                P["wo8h"] = latep.tile([128, H, DM], F8, tag="wo8h",
                                       bufs=1, name="wo8h")
                P["wo8l"] = latep.tile([128, H, DM], F8, tag="wo8l",
                                       bufs=1, name="wo8l")
                nc.sync.dma_start(P["wo8h"][:], wo8h_d[:, :])
                nc.sync.dma_start(P["wo8l"][:], wo8l_d[:, :])
            if c + 1 < QC:
                pgs[c + 1] = G(proj_gen(c + 1))
                filler.append(pgs[c + 1])
            oh8 = otp.tile([128, H, 512], F8, tag="oh8", bufs=3,
                           name=f"oh8_{c}")
            if c == 0:
                ol8 = otp.tile([128, H, 512], F8, tag="ol8", bufs=1)
                os8 = otp.tile([128, H, 512], F8, tag="os8", bufs=1)
            else:
                ol8 = os8 = None
            for h in range(H):
                if c >= 1:
                    # emit the projection prefix this unit's S blocks need
                    gate(pgs.get(c), PJ_NEED[h])
                if c == 0:
                    tail = attn_unit0(h, oh8, ol8, os8, tail)
                else:
                    tail = attn_unit(h, c, oh8, ol8, os8, tail)
                if pending is not None and h == 0:
                    filler.append(G(out_proj_gen(*pending)))
                    pending = None
            pending = (c, oh8, ol8, os8)
        if tail is not None:
            for _ in tail:
                pass
        filler.append(G(out_proj_gen(*pending)))
        pump(10 ** 9)
        P["cm"].__exit__(None, None, None)

    nc.compile()
    return nc


# ---------------------------------------------------------------------------
# Host-side input prep


def _f8_trip(a):
    """hi, lo, hi/32 as fp8 (for the X side)."""
    hi = a.astype(NP_F8)
    lo = (a - hi.astype(np.float32)).astype(NP_F8)
    hs = (hi.astype(np.float32) / WSC).astype(NP_F8)
    return hi, lo, hs


def _w8_pair(a):
    """hi, 32*lo as fp8 (for the W side)."""
    hi = a.astype(NP_F8)
    lo = ((a - hi.astype(np.float32)) * WSC).astype(NP_F8)
    return hi, lo


def make_core_inputs(X, W_qkv, W_out, core, L=L_FULL, DM=DM_FULL,
                     H=H_PER_CORE):
    """Host-side sharding: core -> (batch, head-group) inputs."""
    KT = DM // 128
    QC = L // 512
    b = core // 4
    g = core % 4
    heads = list(range(g * H, (g + 1) * H))

    perm = np.concatenate([np.arange(0, D, 2), np.arange(1, D, 2)])
    w3 = W_qkv.reshape(DM, N_HEADS, 3 * D)
    wq = np.stack([w3[:, h, 0:D][:, perm] for h in heads], axis=1)   # DM H D
    wk = np.stack([w3[:, h, D:2 * D][:, perm] for h in heads], axis=1)
    wv = np.stack([w3[:, h, 2 * D:3 * D] for h in heads], axis=1)
    wo = W_out[g * H * D:(g + 1) * H * D, :]

    out = {}
    # X^T tiles: x8[p, c, j, t] = X[b][512c+t, 128j+p]
    xt = np.ascontiguousarray(X[b].T).astype(np.float32)   # [DM, L]
    xr = xt.reshape(KT, 128, QC, 512).transpose(1, 2, 0, 3)  # p c j t
    xh, xl, xs = _f8_trip(np.ascontiguousarray(xr))
    out["x8h"] = xh
    out["x8l"] = np.ascontiguousarray(xl[:, 0])
    out["x8s"] = np.ascontiguousarray(xs[:, 0])
    for nm, w in (("wq", wq), ("wk", wk)):
        # [DM, H, D] -> [p, h, j, d]
        wr = w.astype(np.float32).reshape(KT, 128, H, D).transpose(1, 2, 0, 3)
        h8, l8 = _w8_pair(np.ascontiguousarray(wr))
        out[nm + "8h"], out[nm + "8l"] = h8, l8
    wvr = wv.reshape(DM, H * D).astype(np.float32)
    wvr = wvr.reshape(KT, 128, H * D).transpose(1, 0, 2)
    out["wv8h"], out["wv8l"] = _w8_pair(np.ascontiguousarray(wvr))
    wor = wo.astype(np.float32).reshape(H, 128, DM).transpose(1, 0, 2)
    out["wo8h"], out["wo8l"] = _w8_pair(np.ascontiguousarray(wor))

    inv_freq = 1.0 / (THETA ** (np.arange(0, D, 2, dtype=np.float32) / D))
    ang = np.arange(L, dtype=np.float32)[:, None] * inv_freq[None, :]
    cos = np.cos(ang).astype(np.float32).T    # [64, L]
    sin = np.sin(ang).astype(np.float32).T
    out["cs1"] = np.ascontiguousarray(np.concatenate([cos, -sin], axis=0))
    out["cs2"] = np.ascontiguousarray(np.concatenate([sin, cos], axis=0))

    pm = np.zeros((128, 2, 128), dtype=np.float32)
    pm[np.arange(128), 0, np.arange(128) % 64] = 1.0
    pm[np.arange(128), 1, 64 + np.arange(128) % 64] = 1.0
    out["pmat"] = pm
    out["pmat8"] = pm.astype(NP_F8)
    kk = np.arange(128)[:, None]
    tt = np.arange(128)[None, :]
    out["mask128"] = (tt >= kk).astype(NP_BF)
    m2 = np.zeros((128, 256), dtype=NP_BF)
    m2[:, 128:] = out["mask128"]
    out["mask256"] = m2
    out["mask128f8"] = out["mask128"].astype(NP_F8)
    out["mask256f8"] = m2.astype(NP_F8)
    out["idm"] = np.eye(128, dtype=np.float32)
    sel = np.zeros((4, 512), dtype=NP_BF)
    for bb in range(4):
        sel[bb, 128 * bb:128 * (bb + 1)] = 1.0
    out["selmat"] = sel
    return out


_NC_CACHE = {}


def get_nc():
    if "nc" not in _NC_CACHE:
        _NC_CACHE["nc"] = build_nc()
    return _NC_CACHE["nc"]


def kernel(X, W_qkv, W_out):
    X = np.asarray(X, dtype=np.float32)
    W_qkv = np.asarray(W_qkv, dtype=np.float32)
    W_out = np.asarray(W_out, dtype=np.float32)
    nc = get_nc()
    # cores c and c+4 share weight shards; trig/pmat/mask are global
    group_maps = [make_core_inputs(X, W_qkv, W_out, g) for g in range(4)]
    xb1 = make_core_inputs(X, W_qkv, W_out, 4)  # batch 1 x8
    in_maps = []
    for c in range(N_CORES):
        m = dict(group_maps[c % 4])
        if c >= 4:
            for k in ("x8h", "x8l", "x8s"):
                m[k] = xb1[k]
        in_maps.append(m)
    res = run_bass_kernel_spmd(nc, in_maps, list(range(N_CORES)))
    out = np.zeros((B_FULL, L_FULL, DM_FULL), dtype=np.float32)
    for c in range(N_CORES):
        out[c // 4] += res.results[c]["y"]
    return out


# revision 46
# speedup vs baseline: 1.0969x; 1.0969x over previous
"""Causal attention with RoPE, tensor-parallel over 8 NeuronCores. v4.

Problem: B=2, L=2048, d_model=2048, H=16 heads, D=128 head dim.
  qkv = X @ W_qkv  (per-head [q|k|v] column layout)
  Q,K rope'd (interleaved pairs), causal softmax(QK^T/sqrt(D)) @ V, @ W_out.

Sharding (Megatron-style): core c handles batch b=c//4 and head group
g=c%4 (4 heads). Each core computes a partial output; host sums 4
partials per batch.

v4 strategy: hybrid precision keyed on causal key count. Softmax-weight
noise from fp8 quantization only survives in rows with few keys (no
averaging), so the first 512-row chunk runs an exact path (3-term fp8
projections, bf16 S/E/O, 3-term out-proj) while chunks 1-3 run an
aggressive path:
  - projections: 1-term plain-fp8 (hi only)
  - S^T blocks: one fp8 DoubleRow matmul per block, stationary =
    (K_hi, zeros) so the dead second plane needs no Q duplication
  - E = exp(S/sqrt(D) - 1.25) stored fp8 (bias keeps exp <= 448 and
    cancels exactly in the softmax ratio)
  - O^T accumulation: fp8 DR over adjacent l-tile pairs (V_hi, E)
  - out-proj: 1-term (O_hi @ Wout_hi)
Numpy-simulated end-to-end metric 1.09e-2 vs the 2e-2 gate.
"""
import math
import numpy as np
import ml_dtypes
import concourse.bacc as bacc
import concourse.mybir as mybir
import concourse.tile as tile
from concourse.bass_utils import run_bass_kernel_spmd

F32 = mybir.dt.float32
F32R = mybir.dt.float32r
BF16 = mybir.dt.bfloat16
F8 = mybir.dt.float8e4
AF = mybir.ActivationFunctionType
DR = mybir.MatmulPerfMode.DoubleRow

N_HEADS = 16
D = 128
THETA = 10000.0
B_FULL, L_FULL, DM_FULL = 2, 2048, 2048
H_PER_CORE = 4
N_CORES = 8
NP_F8 = ml_dtypes.float8_e4m3
NP_BF = ml_dtypes.bfloat16
WSC = 32.0   # W-lo residual pre-scale
EB = -1.25   # exp bias: keeps fp8 E in range; cancels in softmax ratio


def build_nc(L=L_FULL, DM=DM_FULL, H=H_PER_CORE):
    LT = L // 128           # l-tiles (16)
    KT = DM // 128          # contract tiles for projections (16)
    KP = KT // 2            # contract tile PAIRS for DoubleRow (8)
    HD = H * D              # qkv width per core (512)
    QC = L // 512           # q-chunks (4)
    OC = DM // 512          # out-proj n-chunks (4)
    ISQ = 1.0 / math.sqrt(D)

    nc = bacc.Bacc(None, target_bir_lowering=False)

    xh_d = nc.dram_tensor("x8h", [128, QC, KT, 512], F8, kind="ExternalInput")
    xl_d = nc.dram_tensor("x8l", [128, KT, 512], F8, kind="ExternalInput")
    xs_d = nc.dram_tensor("x8s", [128, KT, 512], F8, kind="ExternalInput")
    w8_d = {}
    for w in ("wq", "wk"):
        for p in ("h", "l"):    # hi, 32*lo
            w8_d[w + p] = nc.dram_tensor(f"{w}8{p}", [128, H, KT, D], F8,
                                         kind="ExternalInput")
    for p in ("h", "l"):
        w8_d["wv" + p] = nc.dram_tensor(f"wv8{p}", [128, KT, HD], F8,
                                        kind="ExternalInput")
    cs1_d = nc.dram_tensor("cs1", [128, L], F32, kind="ExternalInput")
    cs2_d = nc.dram_tensor("cs2", [128, L], F32, kind="ExternalInput")
    pmat_d = nc.dram_tensor("pmat", [128, 2, 128], F32R,
                            kind="ExternalInput")
    pmat8_d = nc.dram_tensor("pmat8", [128, 2, 128], F8,
                             kind="ExternalInput")
    mask_d = nc.dram_tensor("mask128", [128, 128], BF16, kind="ExternalInput")
    mask2_d = nc.dram_tensor("mask256", [128, 256], BF16,
                             kind="ExternalInput")
    mask8_d = nc.dram_tensor("mask128f8", [128, 128], F8,
                             kind="ExternalInput")
    mask28_d = nc.dram_tensor("mask256f8", [128, 256], F8,
                              kind="ExternalInput")
    idm_d = nc.dram_tensor("idm", [128, 128], F32, kind="ExternalInput")
    sel_d = nc.dram_tensor("selmat", [4, 512], BF16, kind="ExternalInput")
    wo8h_d = nc.dram_tensor("wo8h", [128, H, DM], F8, kind="ExternalInput")
    wo8l_d = nc.dram_tensor("wo8l", [128, H, DM], F8, kind="ExternalInput")
    y_d = nc.dram_tensor("y", [L, DM], F32, kind="ExternalOutput")

    with tile.TileContext(nc) as tc:
        with tc.tile_pool(name="res", bufs=1) as resp:
            # resident across all phases
            qt8 = resp.tile([128, 2, H, L], F8)     # plane1 = zeros
            kt8 = resp.tile([128, 2, H, L], F8)     # plane1 = zeros
            qtbf = resp.tile([128, H, 512], BF16)   # chunk-0 Q (bf16)
            ktbf = resp.tile([128, H, 512], BF16)   # K tiles 0-3 (bf16)
            v8 = resp.tile([128, LT, HD], F8)       # V [l,d] fp8
            vbf = resp.tile([128, 4, HD], BF16)     # V tiles 0-3 (bf16)
            pmat = resp.tile([128, 2, 128], F32R)
            pmat8 = resp.tile([128, 2, 128], F8)
            mask = resp.tile([128, 128], BF16)
            mask2 = resp.tile([128, 256], BF16)
            mask8 = resp.tile([128, 128], F8)
            mask28 = resp.tile([128, 256], F8)
            idm = resp.tile([128, 128], F32)
            selm = resp.tile([4, 512], BF16)
            ones = resp.tile([128, 1], BF16)
            ones8 = resp.tile([128, 1], F8)
            ebt = resp.tile([128, 1], F32)
            nc.vector.memset(ones[:], 1.0)
            nc.vector.memset(ones8[:], 1.0)
            nc.vector.memset(ebt[:], EB)
            # dead DR planes must be defined (0 * NaN would poison S)
            nc.gpsimd.memset(qt8[:, 1], 0.0)
            nc.gpsimd.memset(kt8[:, 1], 0.0)
            # lc3 X tile + V weights stay alive into phase B: the lc3
            # V-projection units are deferred there as PE filler
            x3h = resp.tile([128, KT, 512], F8)
            wv8h_r = resp.tile([128, KT, HD], F8)

            # ---------------- Phase A ----------------
            with (
                tc.tile_pool(name="w8", bufs=1) as wp,
                tc.tile_pool(name="x8", bufs=2) as xp,
                tc.tile_pool(name="trig", bufs=1) as trp,
                tc.tile_pool(name="uw", bufs=2) as uwp,
                tc.tile_pool(name="psA", bufs=4, space="PSUM") as psA,
                tc.tile_pool(name="psR", bufs=2, space="PSUM") as psR,
            ):
                w8 = {}
                for k in w8_d:
                    if k == "wvh":
                        w8[k] = wv8h_r
                        continue
                    if k == "wvl":
                        w8[k] = wp.tile([128, KT, HD], F8, tag=k, name=k)
                        continue
                    w8[k] = wp.tile([128, H, KT, D], F8, tag=k, name=k)
                cs1 = trp.tile([128, L], F32)
                cs2 = trp.tile([128, L], F32)
                # chunk-0 X: hi/lo/his; chunks 1-3: hi only
                x0l = trp.tile([128, KT, 512], F8)
                x0s = trp.tile([128, KT, 512], F8)
                xh_t = {}
                for lc in range(QC - 1):
                    xh_t[lc] = xp.tile([128, KT, 512], F8, tag="xh",
                                       name=f"xh{lc}")
                xh_t[QC - 1] = x3h
                # DMA order = first-needed-first, alternating SP/ACT
                # dispatch queues so sequencer time doesn't serialize the
                # cold start.
                _alt = [0]

                def dma(dst, src):
                    eng = (nc.sync, nc.scalar)[_alt[0] % 2]
                    _alt[0] += 1
                    eng.dma_start(dst, src)

                for h in range(H):
                    dma(w8["wqh"][:, h], w8_d["wqh"][:, h])
                    dma(xh_t[0][:, 4 * h:4 * (h + 1)],
                        xh_d[:, 0, 4 * h:4 * (h + 1)])
                for q in range(2):
                    dma(x0l[:, 8 * q:8 * (q + 1)],
                        xl_d[:, 8 * q:8 * (q + 1)])
                for h in range(H):
                    dma(w8["wql"][:, h], w8_d["wql"][:, h])
                    dma(x0s[:, 4 * h:4 * (h + 1)],
                        xs_d[:, 4 * h:4 * (h + 1)])
                dma(cs1[:, 0:512], cs1_d[:, 0:512])
                dma(cs2[:, 0:512], cs2_d[:, 0:512])
                dma(pmat[:], pmat_d[:, :])
                dma(pmat8[:], pmat8_d[:, :])
                for h in range(H):
                    dma(w8["wkh"][:, h], w8_d["wkh"][:, h])
                dma(w8["wkl"][:], w8_d["wkl"][:, :])
                dma(w8["wvh"][:], w8_d["wvh"][:, :])
                dma(w8["wvl"][:], w8_d["wvl"][:, :])
                dma(cs1[:, 512:], cs1_d[:, 512:])
                dma(cs2[:, 512:], cs2_d[:, 512:])
                dma(mask[:], mask_d[:, :])
                dma(mask2[:], mask2_d[:, :])
                dma(mask8[:], mask8_d[:, :])
                dma(mask28[:], mask28_d[:, :])
                dma(idm[:], idm_d[:, :])
                dma(selm[:], sel_d[:, :])

                # software pipeline: emit projection matmuls for one unit,
                # then the rope/copy tail of the previous unit
                pend = None   # (kind, h, lc, ps)

                def flush(nxt):
                    nonlocal pend
                    if pend is not None:
                        kind, h, lc, ps = pend
                        if kind == "v":
                            li = h
                            if lc == 0:
                                nc.scalar.activation(vbf[:, li, :], ps[:],
                                                     AF.Copy)
                                nc.scalar.activation(v8[:, li, :], ps[:],
                                                     AF.Copy)
                            else:
                                nc.scalar.activation(v8[:, 4 * lc + li, :],
                                                     ps[:], AF.Copy)
                        elif lc == 0:
                            # exact f32r fold for the chunk-0 (bf16) path
                            u = uwp.tile([128, 512], F32R, tag="u")
                            w = uwp.tile([128, 512], F32R, tag="w")
                            sl = slice(0, 512)
                            nc.vector.tensor_mul(u[:], ps[:], cs1[:, sl])
                            nc.vector.tensor_mul(w[:], ps[:], cs2[:, sl])
                            pr = psR.tile([128, 512], F32, tag="pr")
                            nc.tensor.matmul(pr[:], pmat[:, 0], u[:],
                                             start=True, stop=False,
                                             skip_group_check=True)
                            nc.tensor.matmul(pr[:], pmat[:, 1], w[:],
                                             start=False, stop=True,
                                             skip_group_check=True)
                            if kind == "q":
                                nc.scalar.activation(qtbf[:, h, :],
                                                     pr[:], AF.Copy)
                            else:
                                nc.scalar.activation(ktbf[:, h, :],
                                                     pr[:], AF.Copy)
                                nc.scalar.activation(kt8[:, 0, h, 0:512],
                                                     pr[:], AF.Copy)
                        else:
                            # late chunks: fp8 DR fold (one matmul)
                            uw8 = uwp.tile([128, 2, 512], F8, tag="uw8")
                            sl = slice(512 * lc, 512 * (lc + 1))
                            nc.vector.tensor_mul(uw8[:, 0], ps[:],
                                                 cs1[:, sl])
                            nc.vector.tensor_mul(uw8[:, 1], ps[:],
                                                 cs2[:, sl])
                            pr = psR.tile([128, 512], F32, tag="pr")
                            nc.tensor.matmul(pr[:], pmat8[:, :], uw8[:],
                                             start=True, stop=True,
                                             perf_mode=DR)
                            dst = qt8 if kind == "q" else kt8
                            nc.scalar.activation(dst[:, 0, h, sl],
                                                 pr[:], AF.Copy)
                    pend = nxt

                for lc in range(QC):
                    xh = xh_t[lc]
                    if lc > 0:
                        nc.sync.dma_start(xh[:], xh_d[:, lc])
                    for wn, kind in (("wq", "q"), ("wk", "k")):
                        wh_t, wl_t = w8[wn + "h"], w8[wn + "l"]
                        if lc == 0:
                            terms = ((wh_t, xh), (wh_t, x0l), (wl_t, x0s))
                        else:
                            terms = ((wh_t, xh),)
                        if lc == 0 and wn == "wq":
                            # term-major: fill the PE while late DMAs land.
                            # t1/t2 j-outer (matches X half arrival rate),
                            # t3 h-outer (matches per-head wql arrival).
                            qps = [psA.tile([128, 512], F32, tag="ps",
                                            name=f"q0ps{h}")
                                   for h in range(H)]
                            for t, (ww, xx) in enumerate(terms):
                                hj = ([(h, j) for j in range(KP)
                                       for h in range(H)] if t < 2 else
                                      [(h, j) for h in range(H)
                                       for j in range(KP)])
                                for h, j in hj:
                                    jsl = slice(2 * j, 2 * j + 2)
                                    nc.tensor.matmul(
                                        qps[h][:], ww[:, h, jsl],
                                        xx[:, jsl],
                                        start=(t == 0 and j == 0),
                                        stop=(t == 2 and j == KP - 1),
                                        perf_mode=DR)
                            for h in range(H):
                                flush((kind, h, lc, qps[h]))
                            continue
                        nt = len(terms)
                        for h in range(H):
                            ps = psA.tile([128, 512], F32, tag="ps")
                            for t, (ww, xx) in enumerate(terms):
                                for j in range(KP):
                                    jsl = slice(2 * j, 2 * j + 2)
                                    nc.tensor.matmul(
                                        ps[:], ww[:, h, jsl], xx[:, jsl],
                                        start=(t == 0 and j == 0),
                                        stop=(t == nt - 1 and j == KP - 1),
                                        perf_mode=DR)
                            flush((kind, h, lc, ps))
                    # V for the 4 l-tiles of this chunk: stationary = X
                    # tile. lc3's units are deferred into phase B.
                    wvh4, wvl4 = w8["wvh"], w8["wvl"]
                    for li in ([] if lc == QC - 1 else range(4)):
                        ps = psA.tile([128, 512], F32, tag="ps")
                        lsl = slice(128 * li, 128 * (li + 1))
                        if lc == 0:
                            vterms = ((xh, wvh4), (x0l, wvh4), (x0s, wvl4))
                        else:
                            vterms = ((xh, wvh4),)
                        nt = len(vterms)
                        for t, (xx, ww) in enumerate(vterms):
                            for j in range(KP):
                                jsl = slice(2 * j, 2 * j + 2)
                                nc.tensor.matmul(
                                    ps[:], xx[:, jsl, lsl], ww[:, jsl],
                                    start=(t == 0 and j == 0),
                                    stop=(t == nt - 1 and j == KP - 1),
                                    perf_mode=DR)
                        flush(("v", li, lc, ps))
                flush(None)

            # ---------------- Phase B + C ----------------
            with (
                tc.tile_pool(name="wo8", bufs=1) as wop,
                tc.tile_pool(name="es", bufs=3) as ep,
                tc.tile_pool(name="esb", bufs=2) as ebp,
                tc.tile_pool(name="ot", bufs=2) as otp,
                tc.tile_pool(name="dn", bufs=2) as dnp,
                tc.tile_pool(name="yst", bufs=4) as yp,
                tc.tile_pool(name="psS", bufs=2, space="PSUM") as psS,
                tc.tile_pool(name="psO", bufs=1, space="PSUM") as psO,
                tc.tile_pool(name="psY", bufs=2, space="PSUM") as psY,
                tc.tile_pool(name="psd", bufs=1, space="PSUM") as psd,
            ):
                wo8h = wop.tile([128, H, DM], F8, tag="wo8h")
                wo8l = wop.tile([128, H, DM], F8, tag="wo8l")
                nc.sync.dma_start(wo8h[:], wo8h_d[:, :])
                nc.sync.dma_start(wo8l[:], wo8l_d[:, :])

                filler = []

                def v3_gen():
                    lc = QC - 1
                    for li in range(4):
                        ps = psY.tile([128, 512], F32, tag="psy",
                                      name=f"v3ps{li}")
                        lsl = slice(128 * li, 128 * (li + 1))
                        for j in range(KP):
                            jsl = slice(2 * j, 2 * j + 2)
                            nc.tensor.matmul(
                                ps[:], x3h[:, jsl, lsl], wv8h_r[:, jsl],
                                start=(j == 0), stop=(j == KP - 1),
                                perf_mode=DR)
                            yield
                        nc.vector.tensor_copy(v8[:, 4 * lc + li, :], ps[:])
                        yield

                def pump(k):
                    n = 0
                    while filler and n < k:
                        try:
                            next(filler[0])
                            n += 1
                        except StopIteration:
                            filler.pop(0)

                def out_proj_gen(c, oh8, ol8, os8):
                    # chunk 0: 3-term; chunks 1-3: 1-term (hi only)
                    if c == 0:
                        yterms = ((oh8, wo8h), (ol8, wo8h), (os8, wo8l))
                    else:
                        yterms = ((oh8, wo8h),)
                    nt = len(yterms)
                    for li in range(4):
                        i = 4 * c + li
                        lsl = slice(128 * li, 128 * (li + 1))
                        ysb = yp.tile([128, OC, 512], F32, tag="ysb")
                        for o in range(OC):
                            osl = slice(512 * o, 512 * (o + 1))
                            psy = psY.tile([128, 512], F32, tag="psy")
                            for t, (oo, ww) in enumerate(yterms):
                                for hp in range(H // 2):
                                    h2 = slice(2 * hp, 2 * hp + 2)
                                    nc.tensor.matmul(
                                        psy[:],
                                        oo[:, h2, lsl],
                                        ww[:, h2, osl],
                                        start=(t == 0 and hp == 0),
                                        stop=(t == nt - 1 and
                                              hp == H // 2 - 1),
                                        perf_mode=DR)
                                    yield
                            if c == QC - 1 and o % 2 == 0:
                                nc.scalar.activation(ysb[:, o, :], psy[:],
                                                     AF.Copy)
                            else:
                                nc.vector.tensor_copy(ysb[:, o, :], psy[:])
                            if c == QC - 1 and li == 3:
                                # last stores: spread across engine DGE
                                # queues so SP dispatch doesn't serialize
                                # the drain
                                eng = (nc.scalar, nc.sync,
                                       nc.scalar, nc.sync)[o]
                                eng.dma_start(
                                    y_d[128 * i:128 * (i + 1), osl],
                                    ysb[:, o, :])
                            elif o % 2 == 1:
                                nc.sync.dma_start(
                                    y_d[128 * i:128 * (i + 1),
                                        512 * (o - 1):512 * (o + 1)],
                                    ysb[:, o - 1:o + 1, :])
                            yield

                def den_tail_gen(pso, psden, h, c, oh8, ol8, os8):
                    # deferred den-chain: PE ops interleave with the NEXT
                    # unit's S blocks so the DVE reciprocal never gates PE
                    inv = dnp.tile([128, 4], F32, tag="inv")
                    nc.vector.reciprocal(inv[:], psden[:])
                    yield
                    pit = psd.tile([4, 128], F32, tag="dn", name="pit")
                    nc.tensor.transpose(pit[:], inv[:], idm[:])
                    invt = dnp.tile([4, 128], BF16, tag="invt")
                    nc.vector.tensor_copy(invt[:], pit[:])
                    yield
                    bc = psd.tile([128, 512], F32, tag="dn", name="bc")
                    for b in range(4):
                        bsl = slice(128 * b, 128 * (b + 1))
                        nc.tensor.matmul(bc[:, bsl], selm[:, bsl], invt[:],
                                         start=True, stop=True)
                    yield
                    ot = otp.tile([128, 512], F32R, tag="ot")
                    nc.vector.tensor_mul(ot[:], pso[:], bc[:])
                    nc.vector.tensor_copy(oh8[:, h, :], ot[:])
                    yield
                    if c == 0:
                        nc.vector.tensor_sub(ol8[:, h, :], ot[:],
                                             oh8[:, h, :])
                        nc.gpsimd.tensor_scalar_mul(os8[:, h, :], ot[:],
                                                    1.0 / WSC)
                        yield

                def drive(tail, n):
                    if tail is not None:
                        try:
                            next(tail)
                            return tail
                        except StopIteration:
                            return None
                    pump(n)
                    return None

                def attn_unit0(h, oh8, ol8, os8, tail):
                    # chunk 0: exact bf16 path
                    es = ebp.tile([128, 4, 512], BF16, tag="esb")
                    psden = psd.tile([128, 4], F32, tag="dn", name="psden")
                    for p in range(2):
                        pss = psS.tile([128, 2, 512], F32, tag="pss")
                        for mm_ in range(2):
                            m = 2 * p + mm_
                            qo = 128 * m if m > 0 else 0
                            nc.tensor.matmul(
                                pss[:, mm_, qo:],
                                ktbf[:, h, 128 * m:128 * (m + 1)],
                                qtbf[:, h, qo:],
                                start=True, stop=True)
                            tail = drive(tail, 4)
                        qp = 256 * p
                        nc.scalar.activation(es[:, 2 * p:2 * p + 2, qp:],
                                             pss[:, :, qp:], AF.Exp,
                                             scale=ISQ)
                        nc.gpsimd.tensor_mul(
                            es[:, 2 * p, qp:qp + 128],
                            es[:, 2 * p, qp:qp + 128], mask[:])
                        nc.gpsimd.tensor_mul(
                            es[:, 2 * p + 1, qp:qp + 256],
                            es[:, 2 * p + 1, qp:qp + 256], mask2[:])
                    while tail is not None:
                        tail = drive(tail, 0)
                    pso = psO.tile([128, 512], F32, tag="pso")
                    for m in range(4):
                        qo = 128 * m if m > 0 else 0
                        nc.tensor.matmul(
                            pso[:, qo:],
                            vbf[:, m, 128 * h:128 * (h + 1)],
                            es[:, m, qo:],
                            start=(m == 0), stop=(m == 3))
                        for b in range(max(0, m), 4):
                            nc.tensor.matmul(
                                psden[:, b:b + 1],
                                es[:, m, 128 * b:128 * (b + 1)],
                                ones[:],
                                start=(m == 0 and b == 0),
                                stop=(m == 3 and b == 3),
                                skip_group_check=True)
                        pump(1)
                    osb = otp.tile([128, 512], F32R, tag="osb")
                    nc.vector.tensor_copy(osb[:], pso[:])
                    return den_tail_gen(osb, psden, h, 0, oh8, ol8, os8)

                def attn_unit(h, c, oh8, ol8, os8, tail):
                    # chunks 1-3: fp8 DR path
                    nblk = 4 * (c + 1)
                    es = ep.tile([128, LT, 512], F8, tag="es")
                    psden = psd.tile([128, 4], F32, tag="dn", name="psden")
                    for jp in range(nblk // 2):
                        pss = psS.tile([128, 2, 512], F32, tag="pss")
                        for jj in range(2):
                            j = 2 * jp + jj
                            m = j - 4 * c
                            qo = 128 * m if m > 0 else 0
                            nc.tensor.matmul(
                                pss[:, jj, qo:],
                                kt8[:, :, h, 128 * j:128 * (j + 1)],
                                qt8[:, :, h, 512 * c + qo:512 * (c + 1)],
                                start=True, stop=True, perf_mode=DR)
                            tail = drive(tail, 5 if jp < nblk // 4 else 2)
                        j0 = 2 * jp
                        m0 = j0 - 4 * c
                        qp = 128 * m0 if m0 > 0 else 0
                        nc.scalar.activation(es[:, j0:j0 + 2, qp:],
                                             pss[:, :, qp:], AF.Exp,
                                             scale=ISQ, bias=ebt[:])
                        if m0 >= 0:
                            nc.gpsimd.tensor_mul(
                                es[:, j0, qp:qp + 128],
                                es[:, j0, qp:qp + 128], mask8[:])
                            nc.gpsimd.tensor_mul(
                                es[:, j0 + 1, qp:qp + 256],
                                es[:, j0 + 1, qp:qp + 256], mask28[:])
                    while tail is not None:
                        tail = drive(tail, 0)
                    # O^T accumulation (fp8 DR over l-tile pairs) +
                    # denominator colsum matmuls
                    pso = psO.tile([128, 512], F32, tag="pso")
                    hsl = slice(128 * h, 128 * (h + 1))
                    for jp in range(nblk // 2):
                        j0 = 2 * jp
                        m0 = j0 - 4 * c
                        qo = 128 * m0 if m0 > 0 else 0
                        nc.tensor.matmul(
                            pso[:, qo:],
                            v8[:, j0:j0 + 2, hsl],
                            es[:, j0:j0 + 2, qo:],
                            start=(jp == 0), stop=(jp == nblk // 2 - 1),
                            perf_mode=DR)
                        for jj in range(2):
                            j = j0 + jj
                            m = j - 4 * c
                            for b in range(max(0, m), 4):
                                nc.tensor.matmul(
                                    psden[:, b:b + 1],
                                    es[:, j, 128 * b:128 * (b + 1)],
                                    ones8[:],
                                    start=(j == 0 and b == max(0, m)),
                                    stop=(j == nblk - 1 and b == 3),
                                    skip_group_check=True)
                        pump(1)
                    osb = otp.tile([128, 512], F32R, tag="osb")
                    nc.vector.tensor_copy(osb[:], pso[:])
                    return den_tail_gen(osb, psden, h, c, oh8, ol8, os8)

                filler.append(v3_gen())
                pending = None
                tail = None
                for c in range(QC):
                    oh8 = otp.tile([128, H, 512], F8, tag="oh8",
                                   name=f"oh8_{c}")
                    if c == 0:
                        ol8 = otp.tile([128, H, 512], F8, tag="ol8")
                        os8 = otp.tile([128, H, 512], F8, tag="os8")
                    else:
                        ol8 = os8 = None
                    for h in range(H):
                        if c == 0:
                            tail = attn_unit0(h, oh8, ol8, os8, tail)
                        else:
                            tail = attn_unit(h, c, oh8, ol8, os8, tail)
                        if pending is not None and h == 0:
                            filler.append(out_proj_gen(*pending))
                            pending = None
                    pending = (c, oh8, ol8, os8)
                if tail is not None:
                    for _ in tail:
                        pass
                filler.append(out_proj_gen(*pending))
                pump(10 ** 9)

    nc.compile()
    return nc


# ---------------------------------------------------------------------------
# Host-side input prep


def _f8_trip(a):
    """hi, lo, hi/32 as fp8 (for the X side)."""
    hi = a.astype(NP_F8)
    lo = (a - hi.astype(np.float32)).astype(NP_F8)
    hs = (hi.astype(np.float32) / WSC).astype(NP_F8)
    return hi, lo, hs


def _w8_pair(a):
    """hi, 32*lo as fp8 (for the W side)."""
    hi = a.astype(NP_F8)
    lo = ((a - hi.astype(np.float32)) * WSC).astype(NP_F8)
    return hi, lo


def make_core_inputs(X, W_qkv, W_out, core, L=L_FULL, DM=DM_FULL,
                     H=H_PER_CORE):
    """Host-side sharding: core -> (batch, head-group) inputs."""
    KT = DM // 128
    QC = L // 512
    b = core // 4
    g = core % 4
    heads = list(range(g * H, (g + 1) * H))

    perm = np.concatenate([np.arange(0, D, 2), np.arange(1, D, 2)])
    w3 = W_qkv.reshape(DM, N_HEADS, 3 * D)
    wq = np.stack([w3[:, h, 0:D][:, perm] for h in heads], axis=1)   # DM H D
    wk = np.stack([w3[:, h, D:2 * D][:, perm] for h in heads], axis=1)
    wv = np.stack([w3[:, h, 2 * D:3 * D] for h in heads], axis=1)
    wo = W_out[g * H * D:(g + 1) * H * D, :]

    out = {}
    # X^T tiles: x8[p, c, j, t] = X[b][512c+t, 128j+p]
    xt = np.ascontiguousarray(X[b].T).astype(np.float32)   # [DM, L]
    xr = xt.reshape(KT, 128, QC, 512).transpose(1, 2, 0, 3)  # p c j t
    xh, xl, xs = _f8_trip(np.ascontiguousarray(xr))
    out["x8h"] = xh
    out["x8l"] = np.ascontiguousarray(xl[:, 0])
    out["x8s"] = np.ascontiguousarray(xs[:, 0])
    for nm, w in (("wq", wq), ("wk", wk)):
        # [DM, H, D] -> [p, h, j, d]
        wr = w.astype(np.float32).reshape(KT, 128, H, D).transpose(1, 2, 0, 3)
        h8, l8 = _w8_pair(np.ascontiguousarray(wr))
        out[nm + "8h"], out[nm + "8l"] = h8, l8
    wvr = wv.reshape(DM, H * D).astype(np.float32)
    wvr = wvr.reshape(KT, 128, H * D).transpose(1, 0, 2)
    out["wv8h"], out["wv8l"] = _w8_pair(np.ascontiguousarray(wvr))
    wor = wo.astype(np.float32).reshape(H, 128, DM).transpose(1, 0, 2)
    out["wo8h"], out["wo8l"] = _w8_pair(np.ascontiguousarray(wor))

    inv_freq = 1.0 / (THETA ** (np.arange(0, D, 2, dtype=np.float32) / D))
    ang = np.arange(L, dtype=np.float32)[:, None] * inv_freq[None, :]
    cos = np.cos(ang).astype(np.float32).T    # [64, L]
    sin = np.sin(ang).astype(np.float32).T
    out["cs1"] = np.ascontiguousarray(np.concatenate([cos, -sin], axis=0))
    out["cs2"] = np.ascontiguousarray(np.concatenate([sin, cos], axis=0))

    pm = np.zeros((128, 2, 128), dtype=np.float32)
    pm[np.arange(128), 0, np.arange(128) % 64] = 1.0
    pm[np.arange(128), 1, 64 + np.arange(128) % 64] = 1.0
    out["pmat"] = pm
    out["pmat8"] = pm.astype(NP_F8)
    kk = np.arange(128)[:, None]
    tt = np.arange(128)[None, :]
    out["mask128"] = (tt >= kk).astype(NP_BF)
    m2 = np.zeros((128, 256), dtype=NP_BF)
    m2[:, 128:] = out["mask128"]
    out["mask256"] = m2
    out["mask128f8"] = out["mask128"].astype(NP_F8)
    out["mask256f8"] = m2.astype(NP_F8)
    out["idm"] = np.eye(128, dtype=np.float32)
    sel = np.zeros((4, 512), dtype=NP_BF)
    for bb in range(4):
        sel[bb, 128 * bb:128 * (bb + 1)] = 1.0
    out["selmat"] = sel
    return out


_NC_CACHE = {}


def get_nc():
    if "nc" not in _NC_CACHE:
        _NC_CACHE["nc"] = build_nc()
    return _NC_CACHE["nc"]


def kernel(X, W_qkv, W_out):
    X = np.asarray(X, dtype=np.float32)
    W_qkv = np.asarray(W_qkv, dtype=np.float32)
    W_out = np.asarray(W_out, dtype=np.float32)
    nc = get_nc()
    # cores c and c+4 share weight shards; trig/pmat/mask are global
    group_maps = [make_core_inputs(X, W_qkv, W_out, g) for g in range(4)]
    xb1 = make_core_inputs(X, W_qkv, W_out, 4)  # batch 1 x8
    in_maps = []
    for c in range(N_CORES):
        m = dict(group_maps[c % 4])
        if c >= 4:
            for k in ("x8h", "x8l", "x8s"):
                m[k] = xb1[k]
        in_maps.append(m)
    res = run_bass_kernel_spmd(nc, in_maps, list(range(N_CORES)))
    out = np.zeros((B_FULL, L_FULL, DM_FULL), dtype=np.float32)
    for c in range(N_CORES):
        out[c // 4] += res.results[c]["y"]
    return out
